# revision 27
# baseline (speedup 1.0000x reference)
"""Distributed GAT (nn_AdjGAT) kernel for 8 TRN2 NeuronCores — v2.

Math (per-edge softmax logit depends only on the source node):
    w = exp(attn),  head[h,v] = (sum_k w[h,n_k] t[h,n_k]) / (sum_k w[h,n_k])
    out = relu(mean_h(head) + mean_h(b))

The wall is SWDGE descriptor generation for dma_gather on the GpSimd Q7
(~8.6ns/index single-queue, 40960 edge-rows/core, serial).  vs the v1
baseline (515us -> ~406us):
  * ALL 40 gather ops are prepare_only'd up front across 4 SWDGE queues
    (per-queue ring = dynamic_dma_scratch/16 bytes at ~1B/idx; 24
    untriggered ops proven safe) so descgen runs from t~=22us, hiding
    phase 1 + both AllGathers; chunks fire via trigger_dma as the table
    lands (collectives must precede all untriggered preps - ucode rule).
  * gather slots are (partition = node, group = k): the K=16 reduction is
    pure tile-adds - split: groups 8-15 summed on PE (eye-stationary PSUM
    accumulation), groups 0-7 as a DVE bf16 pairwise tree, merged by one
    more PE matmul.  No blk/sel machinery, no PE transposes.
  * denominator = DVE strided tensor_reduce over the staged w columns;
    zero-row w = 1e-30 so no max/guard is needed; per-head 1/den scaling
    runs on ACT (Copy with per-partition scale AP), final head-sum + bias
    on PE into ps2, relu on ACT.
  * phase 1 is node-major: stationary x-block matmuls produce t and attn
    directly in [node, :] layout; params are host-cast to bf16 (HWDGE
    engines cannot cast).
"""

import math
from contextlib import ExitStack

import numpy as np

import concourse.bass as bass
import concourse.bacc as bacc
import concourse.mybir as mybir
from concourse import library_config
from concourse.bass_utils import run_bass_kernel_spmd

F32 = mybir.dt.float32
BF16 = mybir.dt.bfloat16
I16 = mybir.dt.int16

V, D, K, O, H = 20000, 256, 16, 128, 4
NCORES = 8


class Cfg:
    def __init__(self):
        self.V, self.D, self.K, self.O, self.H = V, D, K, O, H
        self.ncores = NCORES
        self.VP = V // NCORES          # 2500 dst nodes per core
        self.DC = D // 128             # 2
        self.SW = H * O                # 512 payload cols
        self.RW = 640                  # table row pitch (elems bf16, 1280B)
        self.RC = self.SW + H          # 516 meaningful cols per row
        self.NCH = self.VP // 128 + (1 if self.VP % 128 else 0)   # 20 chunks
        self.NB = 20                   # 128-node phase-1 blocks (ceil 2500/128)
        self.ZPAD = 12
        self.VPAD = self.VP + self.ZPAD            # 2512
        self.VPH1 = 1280                           # half-1 rows (blocks 0-9)
        self.VPH2 = self.VPAD - self.VPH1          # 1232 (blocks 10-19 + zpad)
        self.VT = self.VPAD * NCORES               # 20096 table rows
        self.VPF = 2560                            # padded xT cols (20*128)
        self.NGT = 3                               # gtile buffers
        self.TRIG0 = 11                            # first trigger after this prep


def build_graph(cfg: Cfg):
    nc = bacc.Bacc(dynamic_dma_scratch_size=98304, num_swdge_queues=4)
    VP, DC, SW, RW, RC = cfg.VP, cfg.DC, cfg.SW, cfg.RW, cfg.RC
    NCH, NB, NGT = cfg.NCH, cfg.NB, cfg.NGT
    VPH1, VPH2, VPAD, VT, VPF = cfg.VPH1, cfg.VPH2, cfg.VPAD, cfg.VT, cfg.VPF
    H_, O_ = cfg.H, cfg.O
    NC = cfg.ncores

    xT = nc.declare_dram_parameter("xT", [cfg.D, VP], BF16, isOutput=False)
    Wm = nc.declare_dram_parameter("Wm", [DC, 128, SW], BF16, isOutput=False)
    WTp = nc.declare_dram_parameter("WT", [O_, H_, DC, 128], BF16, isOutput=False)
    aTp = nc.declare_dram_parameter("aT", [O_, H_], BF16, isOutput=False)
    bp = nc.declare_dram_parameter("b", [H_, O_], BF16, isOutput=False)
    idxp = nc.declare_dram_parameter("idx", [128, NCH * 128], I16, isOutput=False)
    eyep = nc.declare_dram_parameter("eye", [128, 128], BF16, isOutput=False)
    out_ext = nc.declare_dram_parameter("out", [VP, O_], F32, isOutput=True)

    tbl_loc = nc.dram_tensor("tbl_loc", [VPAD, RW], BF16)
    tbl = nc.dram_tensor("tbl", [VT + 1, RW], BF16, addr_space="Shared")

    ctx = ExitStack()
    sb = lambda name, shape, dt: ctx.enter_context(nc.sbuf_tensor(name, shape, dt))
    xT_sb = sb("xT_sb", [128, DC, VPF], BF16)
    W_sb = sb("W_sb", [128, DC, SW], BF16)
    WT_sb = sb("WT_sb", [128, H_, DC, 128], BF16)
    aT_sb = sb("aT_sb", [128, H_], BF16)
    a3_sb = sb("a3_sb", [128, DC * H_], BF16)
    b_sb = sb("b_sb", [H_, O_], BF16)
    ones4 = sb("ones4", [H_, O_], BF16)
    eye_sb = sb("eye_sb", [128, 128], BF16)
    idx_sb = sb("idx_sb", [128, NCH * 128], I16)
    zero_sb = sb("zero_sb", [cfg.ZPAD, RW], BF16)
    w_sb = [sb(f"w_sb{i}", [128, H_], F32) for i in range(2)]
    stage = [sb(f"stage{i}", [128, RC], BF16) for i in range(2)]
    gtile = [sb(f"gtile{i}", [128, 16, RW], BF16) for i in range(NGT)]
    den_sb = [sb(f"den{i}", [128, H_], F32) for i in range(2)]
    rcp_sb = [sb(f"rcp{i}", [128, H_], F32) for i in range(2)]
    tmp_sb = [sb(f"tmp{i}", [128, SW], BF16) for i in range(2)]
    binit_sb = sb("binit_sb", [128, O_], F32)
    sum_sb = [sb(f"sum{i}", [128, 2, O_], F32) for i in range(2)]
    acc2 = [sb(f"acc2_{i}", [128, O_], F32) for i in range(2)]
    ostage = [sb(f"ostage{i}", [128, O_], F32) for i in range(2)]

    ph1 = ExitStack()
    psa = lambda name, shape, dt: ph1.enter_context(nc.psum_tensor(name, shape, dt))
    ps_t = [psa(f"ps_t{i}", [128, SW], F32) for i in range(2)]
    ps_at = [psa(f"ps_at{i}", [128, H_], F32) for i in range(2)]
    ps_a3 = psa("ps_a3", [128, DC * H_], F32)
    ph1.close()          # phase-1 psum freed; phase-2 banks alias these
    ph2 = ExitStack()
    ps_red = [ph2.enter_context(nc.psum_tensor(f"ps_red{i}", [128, SW], F32))
              for i in range(2)]
    ph2.close()
    ps_bini = nc.psum_tensor("ps_bini", [128, O_], F32).__enter__()

    sctx = ExitStack()
    sem = lambda n: sctx.enter_context(nc.semaphore(n))
    (s_idx, s_ldw, s_ldwt, s_lda, s_ldb, s_dvi, s_zr, s_bini, s_bcp, s_a3,
     s_a3c, s_pt, s_w, s_stg, s_cc, s_prep, s_dv, s_nrm, s_rel, s_mm,
     s_eye, s_rcp, s_tmp, s_ps2) = [
        sem(n) for n in (
            "s_idx", "s_ldw", "s_ldwt", "s_lda", "s_ldb", "s_dvi", "s_zr",
            "s_bini", "s_bcp", "s_a3", "s_a3c", "s_pt", "s_w", "s_stg",
            "s_cc", "s_prep", "s_dv", "s_nrm", "s_rel", "s_mm",
            "s_eye", "s_rcp", "s_tmp", "s_ps2")]
    s_xb = [sem(f"s_xb{i}") for i in range(5)]
    s_sd = [sem("s_sd0"), sem("s_sd1")]
    s_g = [sem(f"s_g{i}") for i in range(NGT)]
    s_o = [sem("s_o0"), sem("s_o1")]

    Exp = mybir.ActivationFunctionType.Exp
    Copy = mybir.ActivationFunctionType.Copy
    Relu = mybir.ActivationFunctionType.Relu
    ADD = mybir.AluOpType.add
    MULT = mybir.AluOpType.mult

    with nc.Block() as block:
        @block.sync
        def _(sy):
            sy.dma_start(out=idx_sb[:, :], in_=idxp[:, :]).then_inc(s_idx, 16)
            sy.dma_start(out=eye_sb[:, :], in_=eyep[:, :]).then_inc(s_eye, 16)
            sy.dma_start(out=b_sb[:, :], in_=bass.AP(
                bp, 0, [[O_, H_], [1, O_]])).then_inc(s_ldb, 16)
            sy.dma_start(out=aT_sb[:, :], in_=bass.AP(
                aTp, 0, [[H_, 128], [1, H_]])).then_inc(s_lda, 16)
            sy.dma_start(out=WT_sb[:, :, :, :], in_=bass.AP(
                WTp, 0, [[H_ * DC * 128, 128], [DC * 128, H_], [128, DC],
                         [1, 128]])).then_inc(s_ldwt, 16)
            sy.dma_start(out=W_sb[:, :, :], in_=bass.AP(
                Wm, 0, [[SW, 128], [128 * SW, DC], [1, SW]])).then_inc(s_ldw, 16)
            for j in range(5):
                lo, hi = j * 512, min(VP, (j + 1) * 512)
                sy.dma_start(out=xT_sb[:, :, lo:hi], in_=bass.AP(
                    xT, lo, [[VP, 128], [128 * VP, DC], [1, hi - lo]])
                ).then_inc(s_xb[j], 16)
            sy.wait_ge(s_dvi, 1)
            sy.dma_start(out=bass.AP(tbl_loc, VP * RW, [[RW, cfg.ZPAD], [1, RW]]),
                         in_=zero_sb[:, :]).then_inc(s_zr, 16)
            for b in range(NB):
                sy.wait_ge(s_stg, b + 1)
                lo = b * 128
                rows = min(128, VP - lo)
                sy.dma_start(
                    out=bass.AP(tbl_loc, lo * RW, [[RW, rows], [1, RC]]),
                    in_=stage[b % 2][0:rows, :]).then_inc(s_sd[b % 2], 16)
            for c in range(NCH):
                sy.wait_ge(s_rel, c + 1)
                lo = c * 128
                rows = min(128, VP - lo)
                sy.dma_start(out=bass.AP(out_ext, lo * O_, [[O_, rows], [1, O_]]),
                             in_=ostage[c % 2][0:rows, :]).then_inc(s_o[c % 2], 16)
            sy.wait_ge(s_o[0], 16 * ((NCH + 1) // 2))
            sy.wait_ge(s_o[1], 16 * (NCH // 2))

        @block.gpsimd
        def _(g):
            g.load_library(library_config.mlp)
            g.wait_ge(s_idx, 16)
            trig = [0]

            def fire(upto):
                while trig[0] <= upto:
                    t = trig[0]
                    g.wait_ge(s_prep, 2 * (t + 1))
                    if t == 0:
                        g.wait_ge(s_cc, 2)
                    if t >= NGT:
                        g.wait_ge(s_mm, t - NGT + 1)
                    g.trigger_dma(count=2, queue_num=t % 4)
                    trig[0] += 1

            for c in range(NCH):
                for hf in range(2):
                    g.dma_gather(
                        out_ap=gtile[c % NGT][:, hf * 8:hf * 8 + 8, :],
                        in_ap=tbl[:, :],
                        idxs_ap=idx_sb[:, c * 128 + hf * 64:c * 128 + hf * 64 + 64],
                        num_idxs=1024,
                        num_idxs_reg=1024,
                        elem_size=RW,
                        prepare_only=True,
                        sem=s_g[c % NGT],
                        queue_num=c % 4,
                    ).then_inc(s_prep, 1)
                if c == 0:
                    g.wait_ge(s_sd[0], 16 * 5)
                    g.wait_ge(s_sd[1], 16 * 5)
                    g.collective_compute(
                        "AllGather", mybir.AluOpType.bypass,
                        replica_groups=[list(range(NC))],
                        ins=[tbl_loc[0:VPH1, :]],
                        outs=[tbl[0:NC * VPH1, :]],
                    ).then_inc(s_cc)
                if c == 1:
                    g.wait_ge(s_sd[0], 16 * 10)
                    g.wait_ge(s_sd[1], 16 * 10)
                    g.wait_ge(s_zr, 16)
                    g.collective_compute(
                        "AllGather", mybir.AluOpType.bypass,
                        replica_groups=[list(range(NC))],
                        ins=[tbl_loc[VPH1:VPAD, :]],
                        outs=[tbl[NC * VPH1:VT, :]],
                    ).then_inc(s_cc)
                if c >= cfg.TRIG0:
                    fire(c - cfg.TRIG0)
            fire(NCH - 1)

        @block.tensor
        def _(pe):
            pe.wait_ge(s_dvi, 1)
            pe.wait_ge(s_ldb, 16)
            pe.matmul(ps_bini[:, :], ones4[:, :], b_sb[:, :],
                      start=True, stop=True).then_inc(s_bini, 1)
            pe.wait_ge(s_ldwt, 16)
            pe.wait_ge(s_lda, 16)
            for c in range(DC):
                for h in range(H_):
                    e = pe.matmul(ps_a3[:, c * H_ + h:c * H_ + h + 1],
                                  WT_sb[:, h, c, :], aT_sb[:, h:h + 1],
                                  start=True, stop=True)
            e.then_inc(s_a3, 1)
            pe.wait_ge(s_a3c, 1)
            pe.wait_ge(s_ldw, 16)
            for b in range(NB):
                pe.wait_ge(s_xb[b // 4], 16)
                if b >= 2:
                    pe.wait_ge(s_stg, b - 1)     # ps_t[b%2] free (ACT read)
                    pe.wait_ge(s_w, b - 1)       # ps_at[b%2] free (ACT read)
                lo = b * 128
                pe.matmul(ps_t[b % 2][:, :], xT_sb[:, 0, lo:lo + 128],
                          W_sb[:, 0, :], start=True, stop=False)
                pe.matmul(ps_t[b % 2][:, :], xT_sb[:, 1, lo:lo + 128],
                          W_sb[:, 1, :], start=False, stop=True)
                pe.matmul(ps_at[b % 2][:, :], xT_sb[:, 0, lo:lo + 128],
                          a3_sb[:, 0:H_], start=True, stop=False)
                e = pe.matmul(ps_at[b % 2][:, :], xT_sb[:, 1, lo:lo + 128],
                              a3_sb[:, H_:2 * H_], start=False, stop=True)
                e.then_inc(s_pt, 1)
            pe.wait_ge(s_eye, 16)
            for c in range(NCH):
                m = c // NGT + 1
                pe.wait_ge(s_g[c % NGT], 32 * m)
                pe.wait_ge(s_dv, c + 1)          # DVE den done reading gtile
                if c >= 2:
                    pe.wait_ge(s_tmp, c - 1)     # ps_red[c%2] free (ACT read)
                for j in range(16):
                    e = pe.matmul(ps_red[c % 2][:, :], eye_sb[:, :],
                                  gtile[c % NGT][:, j, 0:SW],
                                  start=(j == 0), stop=(j == 15))
                e.then_inc(s_mm, 1)

        @block.scalar
        def _(s):
            s.wait_ge(s_a3, 1)
            s.activation(a3_sb[:, :], ps_a3[:, :], Copy).then_inc(s_a3c, 1)
            s.wait_ge(s_bini, 1)
            s.activation(binit_sb[:, :], ps_bini[:, :], Copy).then_inc(s_bcp, 1)
            for b in range(NB):
                s.wait_ge(s_pt, b + 1)
                if b >= 2:
                    s.wait_ge(s_sd[b % 2], 16 * (b // 2))   # stage[b%2] free
                s.activation(w_sb[b % 2][:, :], ps_at[b % 2][:, :], Exp)
                s.activation(stage[b % 2][:, SW:SW + H_], ps_at[b % 2][:, :],
                             Exp).then_inc(s_w, 1)
                s.drain()
                for h in range(H_):
                    e = s.activation(stage[b % 2][:, h * O_:(h + 1) * O_],
                                     ps_t[b % 2][:, h * O_:(h + 1) * O_],
                                     Copy, scale=w_sb[b % 2][:, h:h + 1])
                e.then_inc(s_stg, 1)
            for c in range(NCH):
                s.wait_ge(s_mm, c + 1)           # ps_red[c%2] ready
                s.wait_ge(s_rcp, c + 1)          # rcp[c%2] ready
                for h in range(H_):
                    e = s.activation(tmp_sb[c % 2][:, h * O_:(h + 1) * O_],
                                     ps_red[c % 2][:, h * O_:(h + 1) * O_],
                                     Copy, scale=rcp_sb[c % 2][:, h:h + 1])
                e.then_inc(s_tmp, 1)
                s.wait_ge(s_nrm, c + 1)          # acc2[c%2] summed (DVE)
                if c >= 2:
                    s.wait_ge(s_o[c % 2], 16 * (c // 2))    # ostage free
                s.activation(ostage[c % 2][:, :], acc2[c % 2][:, :],
                             Relu, scale=1.0 / H_).then_inc(s_rel, 1)

        @block.vector
        def _(v):
            v.memset(ones4[:, :], 1.0)
            v.memset(zero_sb[:, :], 0.0)
            v.drain()
            v.memset(zero_sb[:, SW:SW + H_], 1e-30)
            v.memset(xT_sb[:, :, VP:VPF], 0.0).then_inc(s_dvi, 1)
            for c in range(NCH):
                gt = gtile[c % NGT]
                m = c // NGT + 1
                v.wait_ge(s_g[c % NGT], 32 * m)
                if c >= 2:
                    v.wait_ge(s_tmp, c - 1)      # rcp[c%2] free (ACT read)
                # denominator: f32 reduce of the 16 w-slots per head
                e = v.tensor_reduce(
                    den_sb[c % 2][:, :],
                    bass.AP(gt, SW, [[16 * RW, 128], [1, H_], [RW, 16]]),
                    mybir.AxisListType.X, ADD)
                e.then_inc(s_dv, 1)              # gtile free (DVE side)
                v.drain()
                v.reciprocal(rcp_sb[c % 2][:, :],
                             den_sb[c % 2][:, :]).then_inc(s_rcp, 1)
                v.wait_ge(s_tmp, c + 1)          # tmp[c%2] scaled heads ready
                if c >= 2:
                    v.wait_ge(s_rel, c - 1)      # acc2[c%2] free (ACT read)
                v.wait_ge(s_bcp, 1)
                v.tensor_tensor(sum_sb[c % 2][:, 0, :], tmp_sb[c % 2][:, 0:O_],
                                tmp_sb[c % 2][:, O_:2 * O_], ADD)
                v.tensor_tensor(sum_sb[c % 2][:, 1, :],
                                tmp_sb[c % 2][:, 2 * O_:3 * O_],
                                tmp_sb[c % 2][:, 3 * O_:4 * O_], ADD)
                v.drain()
                v.scalar_tensor_tensor(acc2[c % 2][:, :], sum_sb[c % 2][:, 0, :],
                                       0.0, sum_sb[c % 2][:, 1, :], ADD, ADD)
                v.drain()
                e = v.tensor_tensor(acc2[c % 2][:, :], acc2[c % 2][:, :],
                                    binit_sb[:, :], ADD)
                e.then_inc(s_nrm, 1)

    sctx.close()
    ctx.close()
    nc.compile()
    return nc


def prep_core_inputs(cfg: Cfg, x, W, a, b, adj_lst, r):
    """Host-side shard/layout prep for core r (index/layout only, no math)."""
    VP, NCH, NC = cfg.VP, cfg.NCH, cfg.ncores
    H1, H2 = cfg.VPH1, cfg.VPH2
    import ml_dtypes
    bf = ml_dtypes.bfloat16
    xs = np.ascontiguousarray(x[r * VP:(r + 1) * VP].T).astype(bf)
    adj = np.asarray(adj_lst[r * VP:(r + 1) * VP])
    # table row ids for the 2-half AllGather layout
    rr, q = adj // VP, adj % VP
    rows = np.where(q < H1, rr * H1 + q, NC * H1 + rr * H2 + (q - H1))
    zrow = NC * H1 + (VP - H1)                   # first zero-pad row (core 0)
    rows = np.where(adj == cfg.V, zrow, rows).astype(np.int32)
    rows_p = np.full((NCH * 128, cfg.K), zrow, np.int32)
    rows_p[:VP] = rows
    # gtile[p, k, :] = tbl[flat[k*128+p]]; flat j read from idxs[j%16, j//16]
    arr = rows_p.reshape(NCH, 128, cfg.K).transpose(0, 2, 1)   # [c, k, p]
    F = arr.reshape(NCH, 2, 64, 16)              # [c, half, s, q]: j = s*16+q
    B = F.transpose(0, 1, 3, 2)                  # [c, half, q, s]
    idx = np.ascontiguousarray(
        np.tile(B, (1, 1, 8, 1)).transpose(2, 0, 1, 3).reshape(128, NCH * 128)
    ).astype(np.int16)
    W_ = np.asarray(W, np.float32)
    Wm = np.ascontiguousarray(
        W_.transpose(1, 0, 2).reshape(cfg.D, cfg.H * cfg.O)
        .reshape(cfg.DC, 128, cfg.H * cfg.O))
    WT = np.ascontiguousarray(
        W_.transpose(2, 0, 1).reshape(cfg.O, cfg.H, cfg.DC, 128))
    return {
        "xT": xs, "Wm": Wm.astype(bf), "WT": WT.astype(bf),
        "aT": np.ascontiguousarray(np.asarray(a, np.float32).T).astype(bf),
        "b": np.asarray(b, np.float32).astype(bf),
        "idx": idx, "eye": np.eye(128, dtype=np.float32).astype(bf),
    }


_GRAPH_CACHE = {}


def kernel(x, W, a, b, adj_lst, mask_index, _cfg=None, _trace=False):
    cfg = _cfg or Cfg()
    x = np.asarray(x)
    adj_lst = np.asarray(adj_lst)
    assert int(mask_index) == cfg.V
    key = (cfg.V, cfg.ncores)
    if key not in _GRAPH_CACHE:
        _GRAPH_CACHE[key] = build_graph(cfg)
    nc = _GRAPH_CACHE[key]
    in_maps = [prep_core_inputs(cfg, x, W, a, b, adj_lst, r)
               for r in range(cfg.ncores)]
    res = run_bass_kernel_spmd(nc, in_maps, list(range(cfg.ncores)),
                               trace=_trace)
    out = np.concatenate([res.results[r]["out"] for r in range(cfg.ncores)], 0)
    kernel._last_exec_ns = res.exec_time_ns
    return out


# revision 28
# speedup vs baseline: 1.0415x; 1.0415x over previous
"""Distributed GAT (nn_AdjGAT) kernel for 8 TRN2 NeuronCores — v2.

Math (per-edge softmax logit depends only on the source node):
    w = exp(attn),  head[h,v] = (sum_k w[h,n_k] t[h,n_k]) / (sum_k w[h,n_k])
    out = relu(mean_h(head) + mean_h(b))

The wall is SWDGE descriptor generation for dma_gather on the GpSimd Q7
(~8.6ns/index single-queue, 40960 edge-rows/core, serial).  vs the v1
baseline (515us -> ~406us):
  * ALL 40 gather ops are prepare_only'd up front across 4 SWDGE queues
    (per-queue ring = dynamic_dma_scratch/16 bytes at ~1B/idx; 24
    untriggered ops proven safe) so descgen runs from t~=22us, hiding
    phase 1 + both AllGathers; chunks fire via trigger_dma as the table
    lands (collectives must precede all untriggered preps - ucode rule).
  * gather slots are (partition = node, group = k): the K=16 reduction is
    pure tile-adds - split: groups 8-15 summed on PE (eye-stationary PSUM
    accumulation), groups 0-7 as a DVE bf16 pairwise tree, merged by one
    more PE matmul.  No blk/sel machinery, no PE transposes.
  * denominator = DVE strided tensor_reduce over the staged w columns;
    zero-row w = 1e-30 so no max/guard is needed; per-head 1/den scaling
    runs on ACT (Copy with per-partition scale AP), final head-sum + bias
    on PE into ps2, relu on ACT.
  * phase 1 is node-major: stationary x-block matmuls produce t and attn
    directly in [node, :] layout; params are host-cast to bf16 (HWDGE
    engines cannot cast).
"""

import math
from contextlib import ExitStack

import numpy as np

import concourse.bass as bass
import concourse.bacc as bacc
import concourse.mybir as mybir
from concourse import library_config
from concourse.bass_utils import run_bass_kernel_spmd

F32 = mybir.dt.float32
BF16 = mybir.dt.bfloat16
I16 = mybir.dt.int16

V, D, K, O, H = 20000, 256, 16, 128, 4
NCORES = 8


class Cfg:
    def __init__(self):
        self.V, self.D, self.K, self.O, self.H = V, D, K, O, H
        self.ncores = NCORES
        self.VP = V // NCORES          # 2500 dst nodes per core
        self.DC = D // 128             # 2
        self.SW = H * O                # 512 payload cols
        self.RW = 640                  # table row pitch (elems bf16, 1280B)
        self.RC = self.SW + H          # 516 meaningful cols per row
        self.NCH = self.VP // 128 + (1 if self.VP % 128 else 0)   # 20 chunks
        self.NB = 20                   # 128-node phase-1 blocks (ceil 2500/128)
        self.ZPAD = 12
        self.VPAD = self.VP + self.ZPAD            # 2512
        self.VPH1 = 512                            # half-1 rows (blocks 0-3)
        self.VPH2 = self.VPAD - self.VPH1          # 2000 (blocks 4-19 + zpad)
        self.VT = self.VPAD * NCORES               # 20096 table rows
        self.VPF = 2560                            # padded xT cols (20*128)
        self.NGT = 3                               # gtile buffers
        self.TRIG0 = 11                            # first trigger after this prep


def build_graph(cfg: Cfg):
    nc = bacc.Bacc(dynamic_dma_scratch_size=98304, num_swdge_queues=4)
    VP, DC, SW, RW, RC = cfg.VP, cfg.DC, cfg.SW, cfg.RW, cfg.RC
    NCH, NB, NGT = cfg.NCH, cfg.NB, cfg.NGT
    VPH1, VPH2, VPAD, VT, VPF = cfg.VPH1, cfg.VPH2, cfg.VPAD, cfg.VT, cfg.VPF
    H_, O_ = cfg.H, cfg.O
    NC = cfg.ncores

    xT = nc.declare_dram_parameter("xT", [cfg.D, VP], BF16, isOutput=False)
    Wm = nc.declare_dram_parameter("Wm", [DC, 128, SW], BF16, isOutput=False)
    WTp = nc.declare_dram_parameter("WT", [O_, H_, DC, 128], BF16, isOutput=False)
    aTp = nc.declare_dram_parameter("aT", [O_, H_], BF16, isOutput=False)
    bp = nc.declare_dram_parameter("b", [H_, O_], BF16, isOutput=False)
    idxp = nc.declare_dram_parameter("idx", [128, NCH * 128], I16, isOutput=False)
    eyep = nc.declare_dram_parameter("eye", [128, 128], BF16, isOutput=False)
    out_ext = nc.declare_dram_parameter("out", [VP, O_], F32, isOutput=True)

    tbl_loc = nc.dram_tensor("tbl_loc", [VPAD, RW], BF16)
    tbl = nc.dram_tensor("tbl", [VT + 1, RW], BF16, addr_space="Shared")

    ctx = ExitStack()
    sb = lambda name, shape, dt: ctx.enter_context(nc.sbuf_tensor(name, shape, dt))
    xT_sb = sb("xT_sb", [128, DC, VPF], BF16)
    W_sb = sb("W_sb", [128, DC, SW], BF16)
    WT_sb = sb("WT_sb", [128, H_, DC, 128], BF16)
    aT_sb = sb("aT_sb", [128, H_], BF16)
    a3_sb = sb("a3_sb", [128, DC * H_], BF16)
    b_sb = sb("b_sb", [H_, O_], BF16)
    ones4 = sb("ones4", [H_, O_], BF16)
    eye_sb = sb("eye_sb", [128, 128], BF16)
    idx_sb = sb("idx_sb", [128, NCH * 128], I16)
    zero_sb = sb("zero_sb", [cfg.ZPAD, RW], BF16)
    w_sb = [sb(f"w_sb{i}", [128, H_], F32) for i in range(2)]
    stage = [sb(f"stage{i}", [128, RC], BF16) for i in range(2)]
    gtile = [sb(f"gtile{i}", [128, 16, RW], BF16) for i in range(NGT)]
    den_sb = [sb(f"den{i}", [128, H_], F32) for i in range(2)]
    rcp_sb = [sb(f"rcp{i}", [128, H_], F32) for i in range(2)]
    tmp_sb = [sb(f"tmp{i}", [128, SW], BF16) for i in range(2)]
    binit_sb = sb("binit_sb", [128, O_], F32)
    sum_sb = [sb(f"sum{i}", [128, 2, O_], F32) for i in range(2)]
    acc2 = [sb(f"acc2_{i}", [128, O_], F32) for i in range(2)]
    ostage = [sb(f"ostage{i}", [128, O_], F32) for i in range(2)]

    ph1 = ExitStack()
    psa = lambda name, shape, dt: ph1.enter_context(nc.psum_tensor(name, shape, dt))
    ps_t = [psa(f"ps_t{i}", [128, SW], F32) for i in range(2)]
    ps_at = [psa(f"ps_at{i}", [128, H_], F32) for i in range(2)]
    ps_a3 = psa("ps_a3", [128, DC * H_], F32)
    ph1.close()          # phase-1 psum freed; phase-2 banks alias these
    ph2 = ExitStack()
    ps_red = [ph2.enter_context(nc.psum_tensor(f"ps_red{i}", [128, SW], F32))
              for i in range(2)]
    ph2.close()
    ps_bini = nc.psum_tensor("ps_bini", [128, O_], F32).__enter__()

    sctx = ExitStack()
    sem = lambda n: sctx.enter_context(nc.semaphore(n))
    (s_idx, s_ldw, s_ldwt, s_lda, s_ldb, s_dvi, s_zr, s_bini, s_bcp, s_a3,
     s_a3c, s_pt, s_w, s_stg, s_cc, s_prep, s_dv, s_nrm, s_rel, s_mm,
     s_eye, s_rcp, s_tmp, s_ps2) = [
        sem(n) for n in (
            "s_idx", "s_ldw", "s_ldwt", "s_lda", "s_ldb", "s_dvi", "s_zr",
            "s_bini", "s_bcp", "s_a3", "s_a3c", "s_pt", "s_w", "s_stg",
            "s_cc", "s_prep", "s_dv", "s_nrm", "s_rel", "s_mm",
            "s_eye", "s_rcp", "s_tmp", "s_ps2")]
    s_xb = [sem(f"s_xb{i}") for i in range(5)]
    s_sd = [sem("s_sd0"), sem("s_sd1")]
    s_g = [sem(f"s_g{i}") for i in range(NGT)]
    s_o = [sem("s_o0"), sem("s_o1")]

    Exp = mybir.ActivationFunctionType.Exp
    Copy = mybir.ActivationFunctionType.Copy
    Relu = mybir.ActivationFunctionType.Relu
    ADD = mybir.AluOpType.add
    MULT = mybir.AluOpType.mult

    with nc.Block() as block:
        @block.sync
        def _(sy):
            sy.dma_start(out=idx_sb[:, :], in_=idxp[:, :]).then_inc(s_idx, 16)
            sy.dma_start(out=eye_sb[:, :], in_=eyep[:, :]).then_inc(s_eye, 16)
            sy.dma_start(out=b_sb[:, :], in_=bass.AP(
                bp, 0, [[O_, H_], [1, O_]])).then_inc(s_ldb, 16)
            sy.dma_start(out=aT_sb[:, :], in_=bass.AP(
                aTp, 0, [[H_, 128], [1, H_]])).then_inc(s_lda, 16)
            sy.dma_start(out=WT_sb[:, :, :, :], in_=bass.AP(
                WTp, 0, [[H_ * DC * 128, 128], [DC * 128, H_], [128, DC],
                         [1, 128]])).then_inc(s_ldwt, 16)
            sy.dma_start(out=W_sb[:, :, :], in_=bass.AP(
                Wm, 0, [[SW, 128], [128 * SW, DC], [1, SW]])).then_inc(s_ldw, 16)
            for j in range(5):
                lo, hi = j * 512, min(VP, (j + 1) * 512)
                sy.dma_start(out=xT_sb[:, :, lo:hi], in_=bass.AP(
                    xT, lo, [[VP, 128], [128 * VP, DC], [1, hi - lo]])
                ).then_inc(s_xb[j], 16)
            sy.wait_ge(s_dvi, 1)
            sy.dma_start(out=bass.AP(tbl_loc, VP * RW, [[RW, cfg.ZPAD], [1, RW]]),
                         in_=zero_sb[:, :]).then_inc(s_zr, 16)
            for b in range(NB):
                sy.wait_ge(s_stg, b + 1)
                lo = b * 128
                rows = min(128, VP - lo)
                sy.dma_start(
                    out=bass.AP(tbl_loc, lo * RW, [[RW, rows], [1, RC]]),
                    in_=stage[b % 2][0:rows, :]).then_inc(s_sd[b % 2], 16)
            for c in range(NCH):
                sy.wait_ge(s_rel, c + 1)
                lo = c * 128
                rows = min(128, VP - lo)
                sy.dma_start(out=bass.AP(out_ext, lo * O_, [[O_, rows], [1, O_]]),
                             in_=ostage[c % 2][0:rows, :]).then_inc(s_o[c % 2], 16)
            sy.wait_ge(s_o[0], 16 * ((NCH + 1) // 2))
            sy.wait_ge(s_o[1], 16 * (NCH // 2))

        @block.gpsimd
        def _(g):
            g.load_library(library_config.mlp)
            g.wait_ge(s_idx, 16)
            trig = [0]

            def fire(upto):
                while trig[0] <= upto:
                    t = trig[0]
                    g.wait_ge(s_prep, 2 * (t + 1))
                    if t == 0:
                        g.wait_ge(s_cc, 2)
                    if t >= NGT:
                        g.wait_ge(s_mm, t - NGT + 1)
                    g.trigger_dma(count=2, queue_num=t % 4)
                    trig[0] += 1

            for c in range(NCH):
                for hf in range(2):
                    g.dma_gather(
                        out_ap=gtile[c % NGT][:, hf * 8:hf * 8 + 8, :],
                        in_ap=tbl[:, :],
                        idxs_ap=idx_sb[:, c * 128 + hf * 64:c * 128 + hf * 64 + 64],
                        num_idxs=1024,
                        num_idxs_reg=1024,
                        elem_size=RW,
                        prepare_only=True,
                        sem=s_g[c % NGT],
                        queue_num=c % 4,
                    ).then_inc(s_prep, 1)
                if c == 0:
                    g.wait_ge(s_sd[0], 16 * 2)
                    g.wait_ge(s_sd[1], 16 * 2)
                    g.collective_compute(
                        "AllGather", mybir.AluOpType.bypass,
                        replica_groups=[list(range(NC))],
                        ins=[tbl_loc[0:VPH1, :]],
                        outs=[tbl[0:NC * VPH1, :]],
                    ).then_inc(s_cc)
                if c == 1:
                    g.wait_ge(s_sd[0], 16 * 10)
                    g.wait_ge(s_sd[1], 16 * 10)
                    g.wait_ge(s_zr, 16)
                    g.collective_compute(
                        "AllGather", mybir.AluOpType.bypass,
                        replica_groups=[list(range(NC))],
                        ins=[tbl_loc[VPH1:VPAD, :]],
                        outs=[tbl[NC * VPH1:VT, :]],
                    ).then_inc(s_cc)
                if c >= cfg.TRIG0:
                    fire(c - cfg.TRIG0)
            fire(NCH - 1)

        @block.tensor
        def _(pe):
            pe.wait_ge(s_dvi, 1)
            pe.wait_ge(s_ldb, 16)
            pe.matmul(ps_bini[:, :], ones4[:, :], b_sb[:, :],
                      start=True, stop=True).then_inc(s_bini, 1)
            pe.wait_ge(s_ldwt, 16)
            pe.wait_ge(s_lda, 16)
            for c in range(DC):
                for h in range(H_):
                    e = pe.matmul(ps_a3[:, c * H_ + h:c * H_ + h + 1],
                                  WT_sb[:, h, c, :], aT_sb[:, h:h + 1],
                                  start=True, stop=True)
            e.then_inc(s_a3, 1)
            pe.wait_ge(s_a3c, 1)
            pe.wait_ge(s_ldw, 16)
            for b in range(NB):
                pe.wait_ge(s_xb[b // 4], 16)
                if b >= 2:
                    pe.wait_ge(s_stg, b - 1)     # ps_t[b%2] free (ACT read)
                    pe.wait_ge(s_w, b - 1)       # ps_at[b%2] free (ACT read)
                lo = b * 128
                pe.matmul(ps_t[b % 2][:, :], xT_sb[:, 0, lo:lo + 128],
                          W_sb[:, 0, :], start=True, stop=False)
                pe.matmul(ps_t[b % 2][:, :], xT_sb[:, 1, lo:lo + 128],
                          W_sb[:, 1, :], start=False, stop=True)
                pe.matmul(ps_at[b % 2][:, :], xT_sb[:, 0, lo:lo + 128],
                          a3_sb[:, 0:H_], start=True, stop=False)
                e = pe.matmul(ps_at[b % 2][:, :], xT_sb[:, 1, lo:lo + 128],
                              a3_sb[:, H_:2 * H_], start=False, stop=True)
                e.then_inc(s_pt, 1)
            pe.wait_ge(s_eye, 16)
            for c in range(NCH):
                m = c // NGT + 1
                pe.wait_ge(s_g[c % NGT], 32 * m)
                pe.wait_ge(s_dv, c + 1)          # DVE den done reading gtile
                if c >= 2:
                    pe.wait_ge(s_tmp, c - 1)     # ps_red[c%2] free (ACT read)
                for j in range(16):
                    e = pe.matmul(ps_red[c % 2][:, :], eye_sb[:, :],
                                  gtile[c % NGT][:, j, 0:SW],
                                  start=(j == 0), stop=(j == 15))
                e.then_inc(s_mm, 1)

        @block.scalar
        def _(s):
            s.wait_ge(s_a3, 1)
            s.activation(a3_sb[:, :], ps_a3[:, :], Copy).then_inc(s_a3c, 1)
            s.wait_ge(s_bini, 1)
            s.activation(binit_sb[:, :], ps_bini[:, :], Copy).then_inc(s_bcp, 1)
            for b in range(NB):
                s.wait_ge(s_pt, b + 1)
                if b >= 2:
                    s.wait_ge(s_sd[b % 2], 16 * (b // 2))   # stage[b%2] free
                s.activation(w_sb[b % 2][:, :], ps_at[b % 2][:, :], Exp)
                s.activation(stage[b % 2][:, SW:SW + H_], ps_at[b % 2][:, :],
                             Exp).then_inc(s_w, 1)
                s.drain()
                for h in range(H_):
                    e = s.activation(stage[b % 2][:, h * O_:(h + 1) * O_],
                                     ps_t[b % 2][:, h * O_:(h + 1) * O_],
                                     Copy, scale=w_sb[b % 2][:, h:h + 1])
                e.then_inc(s_stg, 1)
            for c in range(NCH):
                s.wait_ge(s_mm, c + 1)           # ps_red[c%2] ready
                s.wait_ge(s_rcp, c + 1)          # rcp[c%2] ready
                for h in range(H_):
                    e = s.activation(tmp_sb[c % 2][:, h * O_:(h + 1) * O_],
                                     ps_red[c % 2][:, h * O_:(h + 1) * O_],
                                     Copy, scale=rcp_sb[c % 2][:, h:h + 1])
                e.then_inc(s_tmp, 1)
                s.wait_ge(s_nrm, c + 1)          # acc2[c%2] summed (DVE)
                if c >= 2:
                    s.wait_ge(s_o[c % 2], 16 * (c // 2))    # ostage free
                s.activation(ostage[c % 2][:, :], acc2[c % 2][:, :],
                             Relu, scale=1.0 / H_).then_inc(s_rel, 1)

        @block.vector
        def _(v):
            v.memset(ones4[:, :], 1.0)
            v.memset(zero_sb[:, :], 0.0)
            v.drain()
            v.memset(zero_sb[:, SW:SW + H_], 1e-30)
            v.memset(xT_sb[:, :, VP:VPF], 0.0).then_inc(s_dvi, 1)
            for c in range(NCH):
                gt = gtile[c % NGT]
                m = c // NGT + 1
                v.wait_ge(s_g[c % NGT], 32 * m)
                if c >= 2:
                    v.wait_ge(s_tmp, c - 1)      # rcp[c%2] free (ACT read)
                # denominator: f32 reduce of the 16 w-slots per head
                e = v.tensor_reduce(
                    den_sb[c % 2][:, :],
                    bass.AP(gt, SW, [[16 * RW, 128], [1, H_], [RW, 16]]),
                    mybir.AxisListType.X, ADD)
                e.then_inc(s_dv, 1)              # gtile free (DVE side)
                v.drain()
                v.reciprocal(rcp_sb[c % 2][:, :],
                             den_sb[c % 2][:, :]).then_inc(s_rcp, 1)
                v.wait_ge(s_tmp, c + 1)          # tmp[c%2] scaled heads ready
                if c >= 2:
                    v.wait_ge(s_rel, c - 1)      # acc2[c%2] free (ACT read)
                v.wait_ge(s_bcp, 1)
                v.tensor_tensor(sum_sb[c % 2][:, 0, :], tmp_sb[c % 2][:, 0:O_],
                                tmp_sb[c % 2][:, O_:2 * O_], ADD)
                v.tensor_tensor(sum_sb[c % 2][:, 1, :],
                                tmp_sb[c % 2][:, 2 * O_:3 * O_],
                                tmp_sb[c % 2][:, 3 * O_:4 * O_], ADD)
                v.drain()
                v.scalar_tensor_tensor(acc2[c % 2][:, :], sum_sb[c % 2][:, 0, :],
                                       0.0, sum_sb[c % 2][:, 1, :], ADD, ADD)
                v.drain()
                e = v.tensor_tensor(acc2[c % 2][:, :], acc2[c % 2][:, :],
                                    binit_sb[:, :], ADD)
                e.then_inc(s_nrm, 1)

    sctx.close()
    ctx.close()
    nc.compile()
    return nc


def prep_core_inputs(cfg: Cfg, x, W, a, b, adj_lst, r):
    """Host-side shard/layout prep for core r (index/layout only, no math)."""
    VP, NCH, NC = cfg.VP, cfg.NCH, cfg.ncores
    H1, H2 = cfg.VPH1, cfg.VPH2
    import ml_dtypes
    bf = ml_dtypes.bfloat16
    xs = np.ascontiguousarray(x[r * VP:(r + 1) * VP].T).astype(bf)
    adj = np.asarray(adj_lst[r * VP:(r + 1) * VP])
    # table row ids for the 2-half AllGather layout
    rr, q = adj // VP, adj % VP
    rows = np.where(q < H1, rr * H1 + q, NC * H1 + rr * H2 + (q - H1))
    zrow = NC * H1 + (VP - H1)                   # first zero-pad row (core 0)
    rows = np.where(adj == cfg.V, zrow, rows).astype(np.int32)
    rows_p = np.full((NCH * 128, cfg.K), zrow, np.int32)
    rows_p[:VP] = rows
    # gtile[p, k, :] = tbl[flat[k*128+p]]; flat j read from idxs[j%16, j//16]
    arr = rows_p.reshape(NCH, 128, cfg.K).transpose(0, 2, 1)   # [c, k, p]
    F = arr.reshape(NCH, 2, 64, 16)              # [c, half, s, q]: j = s*16+q
    B = F.transpose(0, 1, 3, 2)                  # [c, half, q, s]
    idx = np.ascontiguousarray(
        np.tile(B, (1, 1, 8, 1)).transpose(2, 0, 1, 3).reshape(128, NCH * 128)
    ).astype(np.int16)
    W_ = np.asarray(W, np.float32)
    Wm = np.ascontiguousarray(
        W_.transpose(1, 0, 2).reshape(cfg.D, cfg.H * cfg.O)
        .reshape(cfg.DC, 128, cfg.H * cfg.O))
    WT = np.ascontiguousarray(
        W_.transpose(2, 0, 1).reshape(cfg.O, cfg.H, cfg.DC, 128))
    return {
        "xT": xs, "Wm": Wm.astype(bf), "WT": WT.astype(bf),
        "aT": np.ascontiguousarray(np.asarray(a, np.float32).T).astype(bf),
        "b": np.asarray(b, np.float32).astype(bf),
        "idx": idx, "eye": np.eye(128, dtype=np.float32).astype(bf),
    }


_GRAPH_CACHE = {}


def kernel(x, W, a, b, adj_lst, mask_index, _cfg=None, _trace=False):
    cfg = _cfg or Cfg()
    x = np.asarray(x)
    adj_lst = np.asarray(adj_lst)
    assert int(mask_index) == cfg.V
    key = (cfg.V, cfg.ncores)
    if key not in _GRAPH_CACHE:
        _GRAPH_CACHE[key] = build_graph(cfg)
    nc = _GRAPH_CACHE[key]
    in_maps = [prep_core_inputs(cfg, x, W, a, b, adj_lst, r)
               for r in range(cfg.ncores)]
    res = run_bass_kernel_spmd(nc, in_maps, list(range(cfg.ncores)),
                               trace=_trace)
    out = np.concatenate([res.results[r]["out"] for r in range(cfg.ncores)], 0)
    kernel._last_exec_ns = res.exec_time_ns
    return out
